# revision 24
# baseline (speedup 1.0000x reference)
"""Trainium2 Bass kernel for nn_KVMem (scatter_memory attention-to-memory).

Computation (per reference):
  q = h.reshape(B,S,8,128); k = keys_w.reshape(32768,8,128)
  w = softmax(einsum('bshd,zhd->bshz', q, k), axis=z)
  out = einsum('bshz,hdz->bshd', w, values_w.reshape(8,128,32768))

Strategy: shard the memory axis z (32768) across 8 cores (4096 each).
All data fp16. Tokens are processed in 512-halves so a score tile is one
PSUM bank, giving a 7-slot scores ring + 1-bank O accumulator (8 banks).
Per (head, tok-half) pass, for each 128-z tile:
  S[z,tok512]  = K_tile.T @ Q_half      (TensorE, N=512)
  P[z,tok512]  = exp(S)                 (ScalarE over 3-4 ring slots/instr)
  O[d,tok512] += V_tile.T @ P           (TensorE, PSUM-accumulated per pass)
  Dseg[slot]  += P                      (VectorE, one wide add per group)
Denominator: Dseg folded per pass on DVE; the final 128-partition z-sum,
the cross-core reduction of (O, D), and the division happen on HOST.
"""

import sys

sys.path.insert(0, "/opt/trn_rl_repo")

import numpy as np
import ml_dtypes

NCORES = 8
MEMDIM, MEMSIZE, NHEADS = 1024, 32768, 8
B, S = 2, 512
TOK = B * S  # 1024
HALF = 512
HD = MEMDIM // NHEADS  # 128
ZL = MEMSIZE // NCORES  # 4096 z per core
ZT = ZL // 128  # 32 z-tiles per core (per head)
NSLOT = 7  # PSUM score ring slots (1 bank each)

# number of exp groups per head evaluated on DVE via the Schraudolph
# int16/bf16 trick instead of ScalarE (rebalancing lever; 0 = exact only)
SCHRAUD_PER_HEAD = 2
SCH_A = np.float32(2**7 / np.log(2))
SCH_B = np.float32(127 * 128 - 7.5)

_compiled = None


def _build():
    import concourse.bass as bass
    import concourse.tile as tile
    from concourse import bacc, mybir

    nc = bacc.Bacc(
        "TRN2", target_bir_lowering=False, debug=False, num_devices=NCORES
    )
    fp16 = mybir.dt.float16
    bf16 = mybir.dt.bfloat16
    i16 = mybir.dt.int16
    f32 = mybir.dt.float32
    ALU = mybir.AluOpType

    qT = nc.dram_tensor("qT", [128, NHEADS * TOK], fp16, kind="ExternalInput").ap()
    kT = nc.dram_tensor("kT", [128, NHEADS * ZL], fp16, kind="ExternalInput").ap()
    vT = nc.dram_tensor(
        "vT", [128, NHEADS * ZT * HD], fp16, kind="ExternalInput"
    ).ap()
    o_out = nc.dram_tensor(
        "o_out", [128, NHEADS * TOK], f32, kind="ExternalOutput"
    ).ap()
    d_out = nc.dram_tensor(
        "d_out", [128, NHEADS * TOK], fp16, kind="ExternalOutput"
    ).ap()

    with tile.TileContext(nc) as tc:
        with (
            tc.tile_pool(name="const", bufs=1) as cpool,
            tc.tile_pool(name="p", bufs=4) as ppool,
            tc.tile_pool(name="dseg", bufs=2) as segpool,
            tc.tile_pool(name="dtmp", bufs=2) as tmppool,
            tc.tile_pool(name="d", bufs=2) as dpool,
            tc.tile_pool(name="osb", bufs=2) as opool,
            tc.tile_pool(name="ps", bufs=1, space=bass.MemorySpace.PSUM) as pspool,
        ):
            q_sb = cpool.tile([128, NHEADS * TOK], fp16, tag="q", name="q_sb")
            k_sb = cpool.tile([128, NHEADS * ZL], fp16, tag="k", name="k_sb")
            v_sb = cpool.tile(
                [128, NHEADS * ZT * HD], fp16, tag="v", name="v_sb"
            )

            o_half = pspool.tile([128, HALF], f32, tag="o", name="o_half")

            def load_head(h, nchunk, k_first=0):
                nc.sync.dma_start(
                    q_sb[:, h * TOK : (h + 1) * TOK], qT[:, h * TOK : (h + 1) * TOK]
                )

                def k_chunk(ch):
                    zlo, zhi = ch * ZL // nchunk, (ch + 1) * ZL // nchunk
                    nc.sync.dma_start(
                        k_sb[:, h * ZL + zlo : h * ZL + zhi],
                        kT[:, h * ZL + zlo : h * ZL + zhi],
                    )

                def v_chunk(ch):
                    zlo, zhi = ch * ZL // nchunk, (ch + 1) * ZL // nchunk
                    alo, ahi = zlo // 128 * HD, zhi // 128 * HD
                    nc.sync.dma_start(
                        v_sb[:, h * ZT * HD + alo : h * ZT * HD + ahi],
                        vT[:, h * ZT * HD + alo : h * ZT * HD + ahi],
                    )

                for ch in range(k_first):
                    k_chunk(ch)
                for ch in range(nchunk):
                    if ch >= k_first:
                        k_chunk(ch)
                    v_chunk(ch)

            load_head(0, 16, k_first=4)
            load_head(1, 2)

            # deferred V-matmul groups (depth 2: exp gets two group-periods
            # of slack before the PE queue blocks on it)
            pending = []

            def consume_group(gh, ghalf, zbase, A, p_sb, pdt):
                p_ap = p_sb.bitcast(bf16) if pdt is bf16 else p_sb
                for t in range(A):
                    zt = zbase + t
                    nc.tensor.matmul(
                        o_half[:],
                        v_sb[:, (gh * ZT + zt) * HD : (gh * ZT + zt + 1) * HD],
                        p_ap[:, t * HALF : (t + 1) * HALF],
                        start=(zt == 0),
                        stop=(zt == ZT - 1),
                    )

            for h in range(NHEADS):
                if h + 2 < NHEADS:
                    load_head(h + 2, 2)
                d_sb = dpool.tile([128, TOK], fp16, tag="d", name="d_sb")
                sch_left = SCHRAUD_PER_HEAD
                for half in range(2):
                    d_seg = segpool.tile(
                        [128, NSLOT * HALF], fp16, tag="dseg", name="d_seg"
                    )
                    tag_used = {4: False, 3: False}
                    zbase = 0
                    for gi, A in enumerate((4, 3, 4, 3, 4, 3, 4, 3, 4)):
                        s_ps = pspool.tile(
                            [128, A * HALF], f32, tag=f"s{A}", name="s_ps"
                        )
                        for t in range(A):
                            zt = zbase + t
                            nc.tensor.matmul(
                                s_ps[:, t * HALF : (t + 1) * HALF],
                                k_sb[
                                    :, h * ZL + zt * 128 : h * ZL + (zt + 1) * 128
                                ],
                                q_sb[
                                    :,
                                    h * TOK + half * HALF : h * TOK
                                    + (half + 1) * HALF,
                                ],
                            )
                        p_sb = ppool.tile([128, 4 * HALF], fp16, tag="p", name="p_sb")
                        # DVE-exp near the pass end: its PSUM-tile WAR
                        # successor (next pass's 2nd s3 group) is ~4 groups
                        # away, so the slower DVE exp never gates scores
                        if sch_left > 0 and gi == 7:
                            sch_left -= 1
                            pdt = bf16
                            nc.vector.tensor_scalar(
                                p_sb.bitcast(i16)[:, : A * HALF],
                                s_ps[:],
                                float(SCH_A),
                                float(SCH_B),
                                ALU.mult,
                                ALU.add,
                            )
                        else:
                            pdt = fp16
                            nc.scalar.activation(
                                p_sb[:, : A * HALF],
                                s_ps[:],
                                mybir.ActivationFunctionType.Exp,
                            )
                        if len(pending) >= 2:
                            consume_group(*pending.pop(0))
                        pending.append((h, half, zbase, A, p_sb, pdt))
                        # D accumulation: one wide op per group; A=4 groups
                        # use segments [0:4), A=3 groups use [4:7).
                        p_ap = p_sb.bitcast(bf16) if pdt is bf16 else p_sb
                        off = 0 if A == 4 else 4 * HALF
                        dst = d_seg[:, off : off + A * HALF]
                        if tag_used[A]:
                            nc.vector.tensor_tensor(
                                dst, dst, p_ap[:, : A * HALF], ALU.add
                            )
                        else:
                            nc.vector.tensor_copy(dst, p_ap[:, : A * HALF])
                            tag_used[A] = True
                        zbase += A
                    # last V-groups of this pass, then drain O promptly (the
                    # next pass's first V-matmul WARs on o_half via this read)
                    while pending:
                        consume_group(*pending.pop(0))
                    out_sb = (
                        opool.tile([128, TOK], f32, tag="osb", name="out_sb")
                        if half == 0
                        else out_sb
                    )
                    nc.vector.tensor_copy(
                        out_sb[:, half * HALF : (half + 1) * HALF], o_half[:]
                    )
                    # fold the 7 d_seg segments into d_sb on GpSimd (off the
                    # DVE critical path; result only feeds the head-end DMA)
                    x = tmppool.tile([128, 3 * HALF], fp16, tag="x", name="x_fold")
                    nc.gpsimd.tensor_tensor(
                        x[:], d_seg[:, 0 : 3 * HALF], d_seg[:, 3 * HALF : 6 * HALF],
                        ALU.add,
                    )
                    y = d_sb[:, half * HALF : (half + 1) * HALF]
                    nc.gpsimd.tensor_tensor(
                        y, x[:, 0:HALF], x[:, HALF : 2 * HALF], ALU.add
                    )
                    nc.gpsimd.tensor_tensor(y, y, x[:, 2 * HALF : 3 * HALF], ALU.add)
                    nc.gpsimd.tensor_tensor(
                        y, y, d_seg[:, 6 * HALF : 7 * HALF], ALU.add
                    )
                nc.sync.dma_start(o_out[:, h * TOK : (h + 1) * TOK], out_sb[:])
                nc.sync.dma_start(d_out[:, h * TOK : (h + 1) * TOK], d_sb[:])

    nc.compile()
    return nc


def _shard_inputs(h, keys_w, values_w):
    hh = h.reshape(TOK, NHEADS, HD)
    qTf = np.ascontiguousarray(hh.transpose(2, 1, 0).reshape(128, NHEADS * TOK))
    qTf = qTf.astype(np.float16)
    in_maps = []
    for c in range(NCORES):
        ks = keys_w[c * ZL : (c + 1) * ZL]  # [ZL, MEMDIM]
        kTc = np.ascontiguousarray(
            ks.reshape(ZL, NHEADS, HD).transpose(2, 1, 0).reshape(128, NHEADS * ZL)
        ).astype(np.float16)
        vs = values_w[:, c * ZL : (c + 1) * ZL]  # [MEMDIM, ZL]
        v4 = vs.reshape(NHEADS, HD, ZT, 128)  # [h, d, zt, p]
        vTc = np.ascontiguousarray(
            v4.transpose(3, 0, 2, 1).reshape(128, NHEADS * ZT * HD)
        ).astype(np.float16)
        in_maps.append({"qT": qTf, "kT": kTc, "vT": vTc})
    return in_maps


def _combine(results):
    o_acc = np.zeros((128, NHEADS, TOK), np.float64)
    den = np.zeros((NHEADS, TOK), np.float64)
    for r in results:
        o_acc += r["o_out"].reshape(128, NHEADS, TOK).astype(np.float64)
        den += r["d_out"].reshape(128, NHEADS, TOK).astype(np.float64).sum(axis=0)
    res = o_acc / den[None, :, :]  # [d, h, t]
    res = res.transpose(2, 1, 0)  # [t, h, d]
    return np.ascontiguousarray(
        res.reshape(TOK, MEMDIM).reshape(B, S, MEMDIM).astype(np.float32)
    )


def kernel(h, keys_w, values_w, _trace=False, _tmpdir=None):
    global _compiled
    if _compiled is None:
        _compiled = _build()
    from concourse import bass_utils

    in_maps = _shard_inputs(
        np.asarray(h, dtype=np.float32),
        np.asarray(keys_w, dtype=np.float32),
        np.asarray(values_w, dtype=np.float32),
    )
    res = bass_utils.run_bass_kernel_spmd(
        _compiled,
        in_maps,
        core_ids=list(range(NCORES)),
        trace=_trace,
        tmpdir=_tmpdir,
    )
    out = _combine(res.results)
    if _trace:
        return out, res
    return out
